# revision 1
# baseline (speedup 1.0000x reference)
"""Trainium2 Bass kernel for nn_NodeNet (GNN message passing).

Reference computation:
    bo = Ro.T @ X            [E, D]   (gather per-edge source feats)
    bi = Ri.T @ X            [E, D]
    mi = (Ri * e.T) @ bo     [N, D]   (edge-weighted scatter-add)
    mo = (Ro * e.T) @ bi     [N, D]
    M  = [mi, mo, X]         [N, 3D]
    y  = sigmoid(tanh(M @ W1 + b1) @ W2 + b2)

Strategy (8 NeuronCores, edge-sharded):
  - Shard the edge axis E across 8 cores (3072 edges each). Each core
    computes partial mi/mo from its edge shard; AllReduce the [8, N]
    partials; every core runs the tiny MLP and writes the full output.
  - The PE contracts over the partition axis, so the gather needs the
    incidence matrices with N on partitions while the scatter needs E on
    partitions. We upload both layouts in fp16 (host casts/transposes),
    which costs the same HBM bytes as a single fp32 read.
  - e-weighting is applied to the small [E, D] gathered tensors, not the
    big matrices:  mi = Ri @ (e * bo),  mo = Ro @ (e * bi).
"""

import os
import numpy as np

N = 8192
E = 24576
D = 4
H = 100
CORES = 8
ESH = E // CORES          # 3072 edges per core
NCH = N // 128            # 64 node chunks (gather contraction steps)
ECH = ESH // 128          # 24 edge chunks per core
NSLAB = 512               # node-slab width for scatter / MLP
NS = N // NSLAB           # 16 node slabs

_last_exec_time_ns = None
_cached = {}
# fp8 storage for one-hot incidence matrices: flipped on after HW validation.
_FP8_VALIDATED = False


def _build(collective: bool = True, phases: int = 4, r_dtype: str = "float16"):
    # phases: 1=gather only, 2=+scatter, 3=+allreduce, 4=+mlp (full)
    # r_dtype: storage dtype for the big incidence matrices. float8e4 is
    # exact for one-hot matrices and halves HBM traffic.
    import concourse.bass as bass
    import concourse.bacc as bacc
    import concourse.mybir as mybir
    import concourse.tile as tile

    f32 = mybir.dt.float32
    f16 = mybir.dt.float16
    fR = getattr(mybir.dt, r_dtype)

    nc = bacc.Bacc(
        "TRN2",
        target_bir_lowering=False,
        debug=False,
        num_devices=CORES if collective else 1,
    )

    Ri_nat = nc.dram_tensor("Ri_nat", [N, ESH], fR, kind="ExternalInput").ap()
    Ro_nat = nc.dram_tensor("Ro_nat", [N, ESH], fR, kind="ExternalInput").ap()
    RiT = nc.dram_tensor("RiT", [ESH, N], fR, kind="ExternalInput").ap()
    RoT = nc.dram_tensor("RoT", [ESH, N], fR, kind="ExternalInput").ap()
    Xg = nc.dram_tensor("Xg", [128, NCH * D], f16, kind="ExternalInput").ap()
    XT = nc.dram_tensor("XT", [D, N], f32, kind="ExternalInput").ap()
    esh = nc.dram_tensor("esh", [128, ECH], f32, kind="ExternalInput").ap()
    W1 = nc.dram_tensor("W1", [3 * D, H], f32, kind="ExternalInput").ap()
    b1 = nc.dram_tensor("b1", [H, 1], f32, kind="ExternalInput").ap()
    W2 = nc.dram_tensor("W2", [H, 1], f32, kind="ExternalInput").ap()
    b2 = nc.dram_tensor("b2", [1, 1], f32, kind="ExternalInput").ap()
    y = nc.dram_tensor("y", [1, N], f32, kind="ExternalOutput").ap()

    with tile.TileContext(nc) as tc:
        with (
            tc.tile_pool(name="const", bufs=1) as const,
            tc.tile_pool(name="gslab", bufs=3) as gslab_pool,
            tc.tile_pool(name="sslab", bufs=2) as sslab_pool,
            tc.tile_pool(name="small", bufs=1) as small,
            tc.tile_pool(name="mlp", bufs=2) as mlp_pool,
            tc.tile_pool(name="psA", bufs=2, space="PSUM") as psA,
            tc.tile_pool(name="psB", bufs=2, space="PSUM") as psB,
            tc.tile_pool(name="dram", bufs=1, space="DRAM") as dram,
        ):
            # ---- resident small tensors ----
            Xg_sb = const.tile([128, NCH * D], f16)
            nc.sync.dma_start(out=Xg_sb[:], in_=Xg[:])
            e_sb = const.tile([128, ECH], f32)
            nc.sync.dma_start(out=e_sb[:], in_=esh[:])
            W1_sb = const.tile([3 * D, H], f32)
            nc.sync.dma_start(out=W1_sb[:], in_=W1[:])
            b1_sb = const.tile([H, 1], f32)
            nc.sync.dma_start(out=b1_sb[:], in_=b1[:])
            W2_sb = const.tile([H, 1], f32)
            nc.sync.dma_start(out=W2_sb[:], in_=W2[:])
            b2_sb = const.tile([1, 1], f32)
            nc.sync.dma_start(out=b2_sb[:], in_=b2[:])

            # M.T rows: 0-3 mi, 4-7 mo, 8-11 X
            MT_sb = small.tile([3 * D, N], f32)
            nc.sync.dma_start(out=MT_sb[2 * D : 3 * D, :], in_=XT[:])

            # ---- phase 1: gather  b = R.T @ X  -> [ESH, D], e on partitions
            # bv tiles: [128, 12] fp16 per edge chunk, zero-padded so the
            # scatter matmul writes disjoint rows of one [12, NSLAB] psum.
            #   bvi[ech][:, 0:4] = e * bo   (pairs with RiT -> mi rows 0-3)
            #   bvo[ech][:, 4:8] = e * bi   (pairs with RoT -> mo rows 4-7)
            bvi = small.tile([128, ECH * 12], f16)
            bvo = small.tile([128, ECH * 12], f16)
            nc.vector.memset(bvi[:], 0.0)
            nc.vector.memset(bvo[:], 0.0)

            # NOTE: start=True clears has_written for the WHOLE psum bank, so
            # independent accumulation regions cannot share a bank across a
            # long accumulation. Instead: one-shot matmuls per n-chunk into a
            # fresh psum tile, accumulated into fp32 SBUF with DVE adds.
            for Rnat, dst, col0, acc_tag in (
                (Ri_nat, bvo, 4, "bacc_i"),
                (Ro_nat, bvi, 0, "bacc_o"),
            ):
                bacc = small.tile([128, ECH * D], f32, tag=acc_tag)
                for nch in range(NCH):
                    slab = gslab_pool.tile([128, ESH], fR, tag="gs")
                    nc.sync.dma_start(
                        out=slab[:], in_=Rnat[nch * 128 : (nch + 1) * 128, :]
                    )
                    bpsum = psA.tile([128, ECH * D], f32, tag="gather_ps")
                    for ech in range(ECH):
                        nc.tensor.matmul(
                            bpsum[:, ech * D : (ech + 1) * D],
                            lhsT=slab[:, ech * 128 : (ech + 1) * 128],
                            rhs=Xg_sb[:, nch * D : (nch + 1) * D],
                            start=True,
                            stop=True,
                        )
                    if nch == 0:
                        nc.vector.tensor_copy(bacc[:], bpsum[:])
                    else:
                        nc.vector.tensor_add(bacc[:], bacc[:], bpsum[:])
                # bv = e * b, cast to fp16
                for ech in range(ECH):
                    nc.vector.tensor_scalar_mul(
                        dst[:, ech * 12 + col0 : ech * 12 + col0 + D],
                        bacc[:, ech * D : (ech + 1) * D],
                        e_sb[:, ech : ech + 1],
                    )

            # ---- phase 2: scatter  miT/moT = bv.T @ RT  -> psum [12, NSLAB]
            RiT3 = RiT.rearrange("(ec p) n -> p ec n", p=128)
            RoT3 = RoT.rearrange("(ec p) n -> p ec n", p=128)
            for ns in range(NS if phases >= 2 else 0):
                mpsum = psB.tile([3 * D, NSLAB], f32, tag="scat_ps")
                first = True
                for RT3, bv, stag in ((RiT3, bvi, "ssi"), (RoT3, bvo, "sso")):
                    tslab = sslab_pool.tile([128, ECH, NSLAB], fR, tag=stag)
                    nc.sync.dma_start(
                        out=tslab[:],
                        in_=RT3[:, :, ns * NSLAB : (ns + 1) * NSLAB],
                    )
                    for ech in range(ECH):
                        nc.tensor.matmul(
                            mpsum[:],
                            lhsT=bv[:, ech * 12 : (ech + 1) * 12],
                            rhs=tslab[:, ech, :],
                            start=first,
                            stop=(bv is bvo and ech == ECH - 1),
                        )
                        first = False
                nc.vector.tensor_copy(
                    MT_sb[0 : 2 * D, ns * NSLAB : (ns + 1) * NSLAB],
                    mpsum[0 : 2 * D, :],
                )

            # ---- phase 3: all-reduce partial mi/mo across the 8 cores ----
            if collective and phases >= 3:
                ar_in = dram.tile([2 * D, N], f32)
                ar_out = dram.tile([2 * D, N], f32, addr_space="Shared")
                nc.gpsimd.dma_start(out=ar_in[:], in_=MT_sb[0 : 2 * D, :])
                nc.gpsimd.collective_compute(
                    "AllReduce",
                    mybir.AluOpType.add,
                    replica_groups=[list(range(CORES))],
                    ins=[ar_in.opt()],
                    outs=[ar_out.opt()],
                )
                nc.gpsimd.dma_start(out=MT_sb[0 : 2 * D, :], in_=ar_out[:])

            # ---- phase 4: MLP  y = sigmoid(tanh(M @ W1 + b1) @ W2 + b2) ----
            for ns in range(NS if phases >= 4 else 0):
                hpsum = psB.tile([H, NSLAB], f32, tag="h_ps")
                nc.tensor.matmul(
                    hpsum[:],
                    lhsT=W1_sb[:],
                    rhs=MT_sb[:, ns * NSLAB : (ns + 1) * NSLAB],
                    start=True,
                    stop=True,
                )
                h_sb = mlp_pool.tile([H, NSLAB], f32, tag="h_sb")
                nc.scalar.activation(
                    h_sb[:], hpsum[:], mybir.ActivationFunctionType.Tanh,
                    bias=b1_sb[:],
                )
                ypsum = psB.tile([1, NSLAB], f32, tag="y_ps")
                nc.tensor.matmul(
                    ypsum[:], lhsT=W2_sb[:], rhs=h_sb[:], start=True, stop=True
                )
                y_sb = mlp_pool.tile([1, NSLAB], f32, tag="y_sb")
                nc.scalar.activation(
                    y_sb[:], ypsum[:], mybir.ActivationFunctionType.Sigmoid,
                    bias=b2_sb[:],
                )
                nc.sync.dma_start(
                    out=y[:, ns * NSLAB : (ns + 1) * NSLAB], in_=y_sb[:]
                )

    nc.compile()
    return nc


def _get_nc(r_dtype: str = "float16"):
    if r_dtype not in _cached:
        _cached[r_dtype] = _build(r_dtype=r_dtype)
    return _cached[r_dtype]


def _is_binary(a, sample=65536):
    flat = a.reshape(-1)
    s = flat[:: max(1, flat.size // sample)]
    if not np.all((s == 0.0) | (s == 1.0)):
        return False
    return bool(np.all((flat == 0.0) | (flat == 1.0)))


def _r_np_dtype(r_dtype: str):
    if r_dtype == "float16":
        return np.float16
    import ml_dtypes
    return ml_dtypes.float8_e4m3


def _prepare_in_maps(X, e, Ri, Ro, W1, b1, W2, b2, r_dtype: str = "float16"):
    X = np.asarray(X, dtype=np.float32)
    e = np.asarray(e, dtype=np.float32)
    W1 = np.asarray(W1, dtype=np.float32)
    b1 = np.asarray(b1, dtype=np.float32)
    W2 = np.asarray(W2, dtype=np.float32)
    b2 = np.asarray(b2, dtype=np.float32)

    rdt = _r_np_dtype(r_dtype)
    Ri16 = np.asarray(Ri, dtype=np.float32).astype(rdt)
    Ro16 = np.asarray(Ro, dtype=np.float32).astype(rdt)
    RiT16 = np.ascontiguousarray(Ri16.T)   # [E, N]
    RoT16 = np.ascontiguousarray(Ro16.T)

    X16 = X.astype(np.float16)
    # Xg[p, nch*D + d] = X[nch*128 + p, d]
    Xg = np.ascontiguousarray(
        X16.reshape(NCH, 128, D).transpose(1, 0, 2).reshape(128, NCH * D)
    )
    XT = np.ascontiguousarray(X.T)         # [D, N] fp32

    b1c = np.ascontiguousarray(b1.reshape(H, 1))
    b2c = np.ascontiguousarray(b2.reshape(1, 1))
    W1c = np.ascontiguousarray(W1)
    W2c = np.ascontiguousarray(W2.reshape(H, 1))

    in_maps = []
    for c in range(CORES):
        sh = slice(c * ESH, (c + 1) * ESH)
        # esh[p, ech] = e[c*ESH + ech*128 + p]
        e_c = np.ascontiguousarray(
            e.reshape(-1)[sh].reshape(ECH, 128).T
        ).astype(np.float32)
        in_maps.append(
            {
                "Ri_nat": np.ascontiguousarray(Ri16[:, sh]),
                "Ro_nat": np.ascontiguousarray(Ro16[:, sh]),
                "RiT": RiT16[sh],
                "RoT": RoT16[sh],
                "Xg": Xg,
                "XT": XT,
                "esh": e_c,
                "W1": W1c,
                "b1": b1c,
                "W2": W2c,
                "b2": b2c,
            }
        )
    return in_maps


def kernel(**inputs) -> np.ndarray:
    global _last_exec_time_ns
    from concourse import bass_utils

    Ri = np.asarray(inputs["Ri"], dtype=np.float32)
    Ro = np.asarray(inputs["Ro"], dtype=np.float32)
    # fp8 storage is exact for one-hot incidence matrices; otherwise fp16.
    if os.environ.get("KERNEL_R_DTYPE"):
        r_dtype = os.environ["KERNEL_R_DTYPE"]
    elif _FP8_VALIDATED and _is_binary(Ri) and _is_binary(Ro):
        r_dtype = "float8e4"
    else:
        r_dtype = "float16"

    nc = _get_nc(r_dtype)
    in_maps = _prepare_in_maps(
        inputs["X"], inputs["e"], Ri, Ro,
        inputs["W1"], inputs["b1"], inputs["W2"], inputs["b2"],
        r_dtype=r_dtype,
    )
    trace = os.environ.get("KERNEL_TRACE", "") == "1"
    res = bass_utils.run_bass_kernel_spmd(
        nc, in_maps, core_ids=list(range(CORES)), trace=trace
    )
    _last_exec_time_ns = res.exec_time_ns
    out = np.asarray(res.results[0]["y"], dtype=np.float32).reshape(N, 1)
    return out

